# revision 12
# baseline (speedup 1.0000x reference)
"""Trainium2 Bass kernel for nn_CrossAttentionInjector.

Data-parallel over batch: one sample per NeuronCore (B=8 on 8 cores).
Per-core pipeline (layouts transposed so contractions sit on partitions):
  selector (f32r, transposed): PT[rel,n] = Wrk @ condT (+brk per-partition),
    nrm2 = ones^T (PT*PT), rn = exp(-0.5 ln nrm2), phatT = PT*rn_bcast,
    S = rowsum(phatT), u[1,n] = S^T phatT  (u == centrality up to const),
    rank via is_gt count against broadcast row, top-k mask -> exp bias
  qT = Wq @ h, KT = Wk @ cond (bk dropped exactly: constant-in-n shift
    cancels in softmax), V (+bv) with appended ones column per head
  per head h (software-pipelined: scores(h) overlaps attnV(h-1)):
    scoresT = KT_h^T-slices @ qT_h  (n on partitions, q free)
    p = exp(0.125*s + maskbias)  on ACT (single table: ln+exp)
    aug[65,512] = [V|1]^T @ p  -> row 64 = softmax denominator
  denominators DMA'd into 32-aligned slots of per-pair staging tiles,
  one batched reciprocal_approx_fast per pair, Pool partition_broadcast
  (32-aligned bases), DVE multiply -> att; out-proj + bias, DMA out.
"""

import numpy as np

B, C, H, W = 8, 256, 32, 32
N = 512
COND = 512
KVD = 512
RD = 64
NH = 8
S = 1024
NEGB = 30000.0
N_CORES = 8

_cache = {}


def _build():
    import concourse.tile as tile
    import concourse.mybir as mybir
    from concourse import bacc
    import contextlib

    f32 = mybir.dt.float32
    f32r = mybir.dt.float32r
    bf16 = mybir.dt.bfloat16
    A = mybir.AluOpType
    AF = mybir.ActivationFunctionType

    nc = bacc.Bacc("TRN2", target_bir_lowering=False, debug=False)

    hS = nc.dram_tensor("hS", [C, S], bf16, kind="ExternalInput").ap()
    condT = nc.dram_tensor("condT", [COND, N], f32r, kind="ExternalInput").ap()
    condTb = nc.dram_tensor("condTb", [COND, N], bf16, kind="ExternalInput").ap()
    maskc = nc.dram_tensor("maskc", [128, 4], f32, kind="ExternalInput").ap()
    WqT = nc.dram_tensor("WqT", [C, KVD], bf16, kind="ExternalInput").ap()
    WkT = nc.dram_tensor("WkT", [COND, KVD], bf16, kind="ExternalInput").ap()
    WvT = nc.dram_tensor("WvT", [COND, KVD], bf16, kind="ExternalInput").ap()
    WoT = nc.dram_tensor("WoT", [KVD, C], bf16, kind="ExternalInput").ap()
    WrkT = nc.dram_tensor("WrkT", [COND, RD], f32r, kind="ExternalInput").ap()
    bqc = nc.dram_tensor("bqc", [128, 4], f32, kind="ExternalInput").ap()
    bvB = nc.dram_tensor("bvB", [128, KVD], f32, kind="ExternalInput").ap()
    brkc = nc.dram_tensor("brkc", [RD, 1], f32, kind="ExternalInput").ap()
    boc = nc.dram_tensor("boc", [128, 2], f32, kind="ExternalInput").ap()
    onesd = nc.dram_tensor("onesd", [RD, 1], f32, kind="ExternalInput").ap()
    ones1 = nc.dram_tensor("ones1", [1, 1], f32, kind="ExternalInput").ap()
    onesr = nc.dram_tensor("onesr", [128, 1], bf16, kind="ExternalInput").ap()
    y = nc.dram_tensor("y", [C, S], f32, kind="ExternalOutput").ap()

    with tile.TileContext(nc) as tc, contextlib.ExitStack() as ctx:
        cons = ctx.enter_context(tc.tile_pool(name="cons", bufs=1))
        work = ctx.enter_context(tc.tile_pool(name="work", bufs=1))
        ppool = ctx.enter_context(tc.tile_pool(name="ppool", bufs=8))
        o65p = ctx.enter_context(tc.tile_pool(name="o65p", bufs=6))
        rpsp = ctx.enter_context(tc.tile_pool(name="rpsp", bufs=4))

        # ---------------- input DMAs (selector deps first) ----------------
        ct_t = [cons.tile([128, N], f32r, tag=f"ct{i}", name=f"ct{i}") for i in range(4)]
        for i in range(4):
            nc.sync.dma_start(ct_t[i][:], condT[128 * i:128 * (i + 1), :])
        wr_t = [cons.tile([128, RD], f32r, tag=f"wr{i}", name=f"wr{i}") for i in range(4)]
        for i in range(4):
            nc.sync.dma_start(wr_t[i][:], WrkT[128 * i:128 * (i + 1), :])
        brkc_t = cons.tile([RD, 1], f32, tag="brkc")
        nc.sync.dma_start(brkc_t[:], brkc)
        onesd_t = cons.tile([RD, 1], f32, tag="onesd")
        nc.sync.dma_start(onesd_t[:], onesd)
        ones1_t = cons.tile([1, 1], f32, tag="ones1")
        nc.sync.dma_start(ones1_t[:], ones1)
        maskc_t = cons.tile([128, 4], f32, tag="maskc")
        nc.sync.dma_start(maskc_t[:], maskc)
        onesr_t = cons.tile([128, 1], bf16, tag="onesr")
        nc.sync.dma_start(onesr_t[:], onesr)
        ctb_t = [cons.tile([128, N], bf16, tag=f"ctb{i}", name=f"ctb{i}") for i in range(4)]
        for i in range(4):
            nc.sync.dma_start(ctb_t[i][:], condTb[128 * i:128 * (i + 1), :])
        wk_t = [cons.tile([128, KVD], bf16, tag=f"wk{i}", name=f"wk{i}") for i in range(4)]
        wv_t = [cons.tile([128, KVD], bf16, tag=f"wv{i}", name=f"wv{i}") for i in range(4)]
        for i in range(4):
            nc.sync.dma_start(wk_t[i][:], WkT[128 * i:128 * (i + 1), :])
            nc.sync.dma_start(wv_t[i][:], WvT[128 * i:128 * (i + 1), :])
        wq_t = [cons.tile([128, KVD], bf16, tag=f"wq{i}", name=f"wq{i}") for i in range(2)]
        for i in range(2):
            nc.sync.dma_start(wq_t[i][:], WqT[128 * i:128 * (i + 1), :])
        h_t = [cons.tile([128, S], bf16, tag=f"h{i}", name=f"h{i}") for i in range(2)]
        for i in range(2):
            nc.sync.dma_start(h_t[i][:], hS[128 * i:128 * (i + 1), :])
        wo_t = [cons.tile([128, C], bf16, tag=f"wo{i}", name=f"wo{i}") for i in range(4)]
        for i in range(4):
            nc.sync.dma_start(wo_t[i][:], WoT[128 * i:128 * (i + 1), :])
        bqc_t = cons.tile([128, 4], f32, tag="bqc")
        nc.sync.dma_start(bqc_t[:], bqc)
        bvB_t = cons.tile([128, KVD], f32, tag="bvB")
        nc.sync.dma_start(bvB_t[:], bvB)
        boc_t = cons.tile([128, 2], f32, tag="boc")
        nc.sync.dma_start(boc_t[:], boc)

        kt_t = [work.tile([128, N], bf16, tag=f"kt{i}", name=f"kt{i}") for i in range(4)]
        v520 = [work.tile([128, 520], bf16, tag=f"v520_{i}", name=f"v520_{i}") for i in range(4)]
        qt_t = [work.tile([128, S], bf16, tag=f"qt{i}", name=f"qt{i}") for i in range(4)]
        att_t = [work.tile([128, S], bf16, tag=f"att{i}", name=f"att{i}") for i in range(4)]
        biasb = work.tile([128, 4], f32, tag="biasb")

        with tc.tile_pool(name="psSel", bufs=2, space="PSUM") as psSel, \
             tc.tile_pool(name="psM", bufs=3, space="PSUM") as psM:
            # ---- selector projection: PT[rel, n] (f32r fast path) ----
            PTp = psSel.tile([RD, N], f32, tag="psSel", name="PTp")
            for cc in range(4):
                nc.tensor.matmul(PTp[:], wr_t[cc][:], ct_t[cc][:],
                                 start=(cc == 0), stop=(cc == 3))
            PT_sb = work.tile([RD, N], f32, tag="PT_sb")
            nc.vector.tensor_scalar(PT_sb[:], PTp[:], brkc_t[:], None, op0=A.add)
            sq = work.tile([RD, N], f32, tag="sq")
            nc.vector.tensor_tensor(sq[:], PT_sb[:], PT_sb[:], op=A.mult)

            # ---- K projection (interleaves with selector DVE/ACT chain) ----
            for kv in range(2):
                ps = psM.tile([128, N], f32, tag="psM", name=f"kps{kv}")
                for cc in range(4):
                    nc.tensor.matmul(ps[:], wk_t[cc][:, 128 * kv:128 * (kv + 1)],
                                     ctb_t[cc][:], start=(cc == 0), stop=(cc == 3))
                nc.vector.tensor_copy(kt_t[kv][:], ps[:])

            # nrm2 = ones^T sq ; rn = exp(-0.5 ln nrm2)
            nrm2p = psSel.tile([1, N], f32, tag="psSel", name="nrm2p")
            nc.tensor.matmul(nrm2p[:], onesd_t[:], sq[:], start=True, stop=True)
            lns = work.tile([1, N], f32, tag="lns")
            nc.scalar.activation(lns[:], nrm2p[:], AF.Ln)
            rn = work.tile([1, N], f32, tag="rn")
            nc.scalar.activation(rn[:], lns[:], AF.Exp, scale=-0.5)
            rnB = work.tile([RD, N], f32, tag="rnB")
            nc.gpsimd.partition_broadcast(rnB[:], rn[:])
            phatT = work.tile([RD, N], f32, tag="phatT")
            nc.vector.tensor_tensor(phatT[:], PT_sb[:], rnB[:], op=A.mult)
            Scol = work.tile([RD, 1], f32, tag="Scol")
            nc.vector.reduce_sum(Scol[:], phatT[:], axis=mybir.AxisListType.X)

            for kv in range(2, 4):
                ps = psM.tile([128, N], f32, tag="psM", name=f"kps{kv}")
                for cc in range(4):
                    nc.tensor.matmul(ps[:], wk_t[cc][:, 128 * kv:128 * (kv + 1)],
                                     ctb_t[cc][:], start=(cc == 0), stop=(cc == 3))
                nc.vector.tensor_copy(kt_t[kv][:], ps[:])

            # u[1,n] = S^T phatT  (centrality up to a constant)
            up = psSel.tile([1, N], f32, tag="psSel", name="up")
            nc.tensor.matmul(up[:], Scol[:], phatT[:], start=True, stop=True)
            u_sb = work.tile([1, N], f32, tag="u_sb")
            nc.vector.tensor_copy(u_sb[:], up[:])
            cB = work.tile([128, N], f32, tag="cB")
            nc.gpsimd.partition_broadcast(cB[:], u_sb[:])

            # ---- V projection ----
            for hh in range(NH):
                for nn_ in range(4):
                    nc.gpsimd.tensor_copy(v520[nn_][:, 65 * hh + 64:65 * hh + 65],
                                          onesr_t[:])
            for nn_ in range(4):
                ps = psM.tile([128, KVD], f32, tag="psM", name=f"vps{nn_}")
                for cc in range(4):
                    nc.tensor.matmul(ps[:], ctb_t[cc][:, 128 * nn_:128 * (nn_ + 1)],
                                     wv_t[cc][:], start=(cc == 0), stop=(cc == 3))
                vview = v520[nn_][:].rearrange("p (h c) -> p h c", c=65)[:, :, 0:64]
                nc.vector.tensor_tensor(vview,
                                        ps[:].rearrange("p (h c) -> p h c", c=64),
                                        bvB_t[:].rearrange("p (h c) -> p h c", c=64),
                                        op=A.add)

            # rank: c transposed to columns via PE transpose, then is_gt counts
            pst = psSel.tile([128, 4], f32, tag="psSel", name="pst")
            for j in range(4):
                nc.tensor.transpose(pst[:, j:j + 1], u_sb[0:1, 128 * j:128 * (j + 1)],
                                    ones1_t[:])
            c4T = work.tile([128, 4], f32, tag="c4T")
            nc.vector.tensor_copy(c4T[:], pst[:])
            cmpd = work.tile([128, N], f32, tag="cmpd")
            rank4 = work.tile([128, 4], f32, tag="rank4")
            for j in range(4):
                nc.vector.tensor_scalar(cmpd[:], cB[:], c4T[:, j:j + 1], 0.0,
                                        op0=A.is_gt, op1=A.add,
                                        accum_out=rank4[:, j:j + 1])
            selm = work.tile([128, 4], f32, tag="selm")
            nc.vector.tensor_scalar(selm[:], rank4[:], 306.5, None, op0=A.is_lt)
            allowed4 = work.tile([128, 4], f32, tag="allowed4")
            nc.vector.tensor_tensor(allowed4[:], selm[:], maskc_t[:], op=A.mult)
            nc.vector.tensor_scalar(biasb[:], allowed4[:], NEGB, NEGB,
                                    op0=A.mult, op1=A.subtract)

            # ---- Q projection ----
            for kv in range(4):
                for sc in range(2):
                    ps = psM.tile([128, 512], f32, tag="psM", name=f"qps{kv}_{sc}")
                    for cc in range(2):
                        nc.tensor.matmul(ps[:], wq_t[cc][:, 128 * kv:128 * (kv + 1)],
                                         h_t[cc][:, 512 * sc:512 * (sc + 1)],
                                         start=(cc == 0), stop=(cc == 1))
                    nc.vector.tensor_scalar(qt_t[kv][:, 512 * sc:512 * (sc + 1)], ps[:],
                                            bqc_t[:, kv:kv + 1], None, op0=A.add)

        # ---------------- attention (scores(h) pipelined with attnV(h-1)) ----
        with tc.tile_pool(name="psS", bufs=2, space="PSUM") as psS, \
             tc.tile_pool(name="psA", bufs=3, space="PSUM") as psA:
            dstage = [work.tile([4, 512], f32, tag=f"dstage{g}", name=f"dstage{g}")
                      for g in range(4)]
            drecip = [work.tile([4, 512], f32, tag=f"drecip{g}", name=f"drecip{g}")
                      for g in range(4)]
            plists = {}
            o65_l = {}

            def scores_exp(h):
                i2 = h // 2
                po = (h % 2) * 64
                plist = []
                for nn_ in range(4):
                    sps_ = psS.tile([128, 1024], f32, tag="psS", name=f"s_{h}_{nn_}")
                    for qc in range(2):
                        nc.tensor.matmul(sps_[:, 512 * qc:512 * (qc + 1)],
                                         kt_t[i2][po:po + 64, 128 * nn_:128 * (nn_ + 1)],
                                         qt_t[i2][po:po + 64, 512 * qc:512 * (qc + 1)],
                                         start=True, stop=True)
                    p_t = ppool.tile([128, 1024], bf16, tag="p", name=f"p_{h}_{nn_}")
                    nc.scalar.activation(p_t[:], sps_[:], AF.Exp,
                                         bias=biasb[:, nn_:nn_ + 1], scale=0.125)
                    plist.append(p_t)
                plists[h] = plist

            def attnv_tail(h):
                i2 = h // 2
                po = (h % 2) * 64
                g = h // 2
                plist = plists.pop(h)
                for qc in range(2):
                    aug = psA.tile([65, 512], f32, tag="psA", name=f"aug_{h}_{qc}")
                    for nn_ in range(4):
                        nc.tensor.matmul(aug[:], v520[nn_][:, 65 * h:65 * h + 65],
                                         plist[nn_][:, 512 * qc:512 * (qc + 1)],
                                         start=(nn_ == 0), stop=(nn_ == 3))
                    o65 = o65p.tile([65, 512], f32, tag="o65", name=f"o65_{h}_{qc}")
                    nc.vector.tensor_copy(o65[:], aug[:])
                    slot = 2 * (h % 2) + qc
                    nc.sync.dma_start(dstage[g][slot:slot + 1, :], o65[64:65, :])
                    o65_l[(h, qc)] = o65
                if h % 2 == 1:
                    nc.vector.reciprocal_approx_fast(drecip[g][:], dstage[g][:])
                    for hh in (h - 1, h):
                        i2h = hh // 2
                        poh = (hh % 2) * 64
                        for qc in range(2):
                            slot = 2 * (hh % 2) + qc
                            rrow = rpsp.tile([1, 512], f32, tag="rrow",
                                             name=f"rrow_{hh}_{qc}")
                            nc.sync.dma_start(rrow[:], drecip[g][slot:slot + 1, :])
                            rps = rpsp.tile([64, 512], f32, tag="rps",
                                            name=f"rps_{hh}_{qc}")
                            nc.gpsimd.partition_broadcast(rps[:], rrow[:])
                            nc.vector.tensor_tensor(
                                att_t[i2h][poh:poh + 64, 512 * qc:512 * (qc + 1)],
                                o65_l.pop((hh, qc))[0:64, :], rps[:], op=A.mult)

            scores_exp(0)
            for h in range(1, NH):
                scores_exp(h)
                attnv_tail(h - 1)
            attnv_tail(NH - 1)

        # ---------------- output projection ----------------
        with tc.tile_pool(name="psB", bufs=2, space="PSUM") as psB:
            outF = [work.tile([128, S], f32, tag=f"outF{i}", name=f"outF{i}")
                    for i in range(2)]
            for ccn in range(2):
                for sc in range(2):
                    ps = psB.tile([128, 512], f32, tag="psB", name=f"ops{ccn}_{sc}")
                    for kv in range(4):
                        nc.tensor.matmul(ps[:], wo_t[kv][:, 128 * ccn:128 * (ccn + 1)],
                                         att_t[kv][:, 512 * sc:512 * (sc + 1)],
                                         start=(kv == 0), stop=(kv == 3))
                    nc.vector.tensor_scalar(outF[ccn][:, 512 * sc:512 * (sc + 1)],
                                            ps[:], boc_t[:, ccn:ccn + 1], None,
                                            op0=A.add)
                    nc.sync.dma_start(y[128 * ccn:128 * (ccn + 1),
                                        512 * sc:512 * (sc + 1)],
                                      outF[ccn][:, 512 * sc:512 * (sc + 1)])

    nc.compile()
    return nc


def _get_nc():
    if "nc" not in _cache:
        _cache["nc"] = _build()
    return _cache["nc"]


def make_in_maps(**inputs):
    import ml_dtypes
    bf = ml_dtypes.bfloat16
    h = np.asarray(inputs["h"], np.float32)
    cond = np.asarray(inputs["cond_feats"], np.float32)
    cmask = np.asarray(inputs["cond_mask"])
    f = np.float32
    shared = {
        "WqT": np.ascontiguousarray(np.asarray(inputs["Wq"], f).T).astype(bf),
        "WkT": np.ascontiguousarray(np.asarray(inputs["Wk"], f).T).astype(bf),
        "WvT": np.ascontiguousarray(np.asarray(inputs["Wv"], f).T).astype(bf),
        "WoT": np.ascontiguousarray(np.asarray(inputs["Wo"], f).T).astype(bf),
        "WrkT": np.ascontiguousarray(np.asarray(inputs["Wrk"], f).T),
        "bqc": np.ascontiguousarray(np.asarray(inputs["bq"], f).reshape(4, 128).T),
        "bvB": np.ascontiguousarray(np.broadcast_to(np.asarray(inputs["bv"], f), (128, KVD))),
        "brkc": np.ascontiguousarray(np.asarray(inputs["brk"], f).reshape(RD, 1)),
        "boc": np.ascontiguousarray(np.asarray(inputs["bo"], f).reshape(2, 128).T),
        "onesd": np.ones((RD, 1), f),
        "ones1": np.ones((1, 1), f),
        "onesr": np.ones((128, 1), bf),
    }
    in_maps = []
    for b in range(B):
        m = dict(shared)
        m["hS"] = np.ascontiguousarray(h[b].reshape(C, S)).astype(bf)
        m["condT"] = np.ascontiguousarray(cond[b].T)
        m["condTb"] = m["condT"].astype(bf)
        m["maskc"] = np.ascontiguousarray(cmask[b].astype(f).reshape(4, 128).T)
        in_maps.append(m)
    return in_maps


def kernel(**inputs):
    from concourse.bass_utils import run_bass_kernel_spmd
    nc = _get_nc()
    in_maps = make_in_maps(**inputs)
    res = run_bass_kernel_spmd(nc, in_maps, core_ids=list(range(N_CORES)))
    return np.stack([res.results[b]["y"].reshape(C, H, W) for b in range(B)])


# revision 15
# speedup vs baseline: 1.0710x; 1.0710x over previous
"""Trainium2 Bass kernel for nn_CrossAttentionInjector.

Data-parallel over batch: one sample per NeuronCore (B=8 on 8 cores).
Per-core pipeline (layouts transposed so contractions sit on partitions):
  selector (f32r proj, transposed): PT[rel,n] = Wrk @ condT (+brk),
    nrm2 = ones^T (PT*PT), rn = sqrt(recip_fast(nrm2)), phatT = PT*rn_bcast,
    S = rowsum(phatT), u[1,n] = S^T phatT  (== centrality up to const),
    c transposed via PE, rank = is_gt count vs broadcast row, mask ->
    exp bias.  All engines are in-order, so the whole selector chain is
    emitted FIRST on DVE/Pool/ACT; projections fill PE meanwhile.
  qT = Wq @ h, KT = Wk @ cond (bk dropped exactly: a per-query shift
    cancels in softmax), V (+bv) with appended ones column per head.
  per head (scores(h) software-pipelined with attnV(h-1)):
    scoresT = KT_h^T @ qT_h (n on partitions), p = exp(s/8 + maskbias),
    aug[65,512] = [V|1]^T p -> row 64 = softmax denominator.
  division tail in bf16: o65 evac bf16, den rows DMA-gathered per pair,
  recip_fast f32, bf16 recip row DMA-staged to base-0, Pool
  partition_broadcast, bf16 DVE multiply (2x mode).
  out-projection shares the PSUM block so kv0-2 chains overlap the last
  pair's divide chain; +bo on DVE, 4-way output DMA.
DMA triggers are spread across sync/gpsimd/scalar queues (each trigger
costs ~0.6us serial on its queue).
"""

import numpy as np

B, C, H, W = 8, 256, 32, 32
N = 512
COND = 512
KVD = 512
RD = 64
NH = 8
S = 1024
NEGB = 30000.0
N_CORES = 8

_cache = {}


def _build():
    import concourse.tile as tile
    import concourse.mybir as mybir
    from concourse import bacc
    import contextlib

    f32 = mybir.dt.float32
    f32r = mybir.dt.float32r
    bf16 = mybir.dt.bfloat16
    A = mybir.AluOpType
    AF = mybir.ActivationFunctionType

    nc = bacc.Bacc("TRN2", target_bir_lowering=False, debug=False)

    hS = nc.dram_tensor("hS", [C, S], bf16, kind="ExternalInput").ap()
    condT = nc.dram_tensor("condT", [COND, N], f32r, kind="ExternalInput").ap()
    condTb = nc.dram_tensor("condTb", [COND, N], bf16, kind="ExternalInput").ap()
    maskc = nc.dram_tensor("maskc", [128, 4], f32, kind="ExternalInput").ap()
    WqT = nc.dram_tensor("WqT", [C, KVD], bf16, kind="ExternalInput").ap()
    WkT = nc.dram_tensor("WkT", [COND, KVD], bf16, kind="ExternalInput").ap()
    WvT = nc.dram_tensor("WvT", [COND, KVD], bf16, kind="ExternalInput").ap()
    WoT = nc.dram_tensor("WoT", [KVD, C], bf16, kind="ExternalInput").ap()
    WrkT = nc.dram_tensor("WrkT", [COND, RD], f32r, kind="ExternalInput").ap()
    bqc = nc.dram_tensor("bqc", [128, 4], f32, kind="ExternalInput").ap()
    bvB = nc.dram_tensor("bvB", [128, KVD], f32, kind="ExternalInput").ap()
    brkc = nc.dram_tensor("brkc", [RD, 1], f32, kind="ExternalInput").ap()
    boc = nc.dram_tensor("boc", [128, 2], f32, kind="ExternalInput").ap()
    onesd = nc.dram_tensor("onesd", [RD, 1], f32, kind="ExternalInput").ap()
    ones1 = nc.dram_tensor("ones1", [1, 1], f32, kind="ExternalInput").ap()
    onesr = nc.dram_tensor("onesr", [128, 1], bf16, kind="ExternalInput").ap()
    y = nc.dram_tensor("y", [C, S], f32, kind="ExternalOutput").ap()

    with tile.TileContext(nc) as tc, contextlib.ExitStack() as ctx:
        cons = ctx.enter_context(tc.tile_pool(name="cons", bufs=1))
        work = ctx.enter_context(tc.tile_pool(name="work", bufs=1))
        ppool = ctx.enter_context(tc.tile_pool(name="ppool", bufs=8))
        o65p = ctx.enter_context(tc.tile_pool(name="o65p", bufs=6))
        rpsp = ctx.enter_context(tc.tile_pool(name="rpsp", bufs=4))

        # ------------- input DMAs, triggers spread across queues -------------
        # sync queue: selector-critical first, then V/out-proj weights
        wr_t = [cons.tile([128, RD], f32r, tag=f"wr{i}", name=f"wr{i}") for i in range(4)]
        for i in range(4):
            nc.sync.dma_start(wr_t[i][:], WrkT[128 * i:128 * (i + 1), :])
        ct_t = [cons.tile([128, N], f32r, tag=f"ct{i}", name=f"ct{i}") for i in range(4)]
        for i in range(4):
            nc.sync.dma_start(ct_t[i][:], condT[128 * i:128 * (i + 1), :])
        brkc_t = cons.tile([RD, 1], f32, tag="brkc")
        nc.sync.dma_start(brkc_t[:], brkc)
        onesd_t = cons.tile([RD, 1], f32, tag="onesd")
        nc.sync.dma_start(onesd_t[:], onesd)
        ones1_t = cons.tile([1, 1], f32, tag="ones1")
        nc.sync.dma_start(ones1_t[:], ones1)
        maskc_t = cons.tile([128, 4], f32, tag="maskc")
        nc.sync.dma_start(maskc_t[:], maskc)
        wv_t = [cons.tile([128, KVD], bf16, tag=f"wv{i}", name=f"wv{i}") for i in range(4)]
        for i in range(4):
            nc.sync.dma_start(wv_t[i][:], WvT[128 * i:128 * (i + 1), :])
        bvB_t = cons.tile([128, KVD], f32, tag="bvB")
        nc.sync.dma_start(bvB_t[:], bvB)
        wo_t = [cons.tile([128, C], bf16, tag=f"wo{i}", name=f"wo{i}") for i in range(4)]
        for i in range(4):
            nc.sync.dma_start(wo_t[i][:], WoT[128 * i:128 * (i + 1), :])
        boc_t = cons.tile([128, 2], f32, tag="boc")
        nc.sync.dma_start(boc_t[:], boc)

        # gpsimd queue: K-proj inputs + ones column source
        onesr_t = cons.tile([128, 1], bf16, tag="onesr")
        nc.gpsimd.dma_start(onesr_t[:], onesr)
        ctb_t = [cons.tile([128, N], bf16, tag=f"ctb{i}", name=f"ctb{i}") for i in range(4)]
        for i in range(4):
            nc.gpsimd.dma_start(ctb_t[i][:], condTb[128 * i:128 * (i + 1), :])
        wk_t = [cons.tile([128, KVD], bf16, tag=f"wk{i}", name=f"wk{i}") for i in range(4)]
        for i in range(4):
            nc.gpsimd.dma_start(wk_t[i][:], WkT[128 * i:128 * (i + 1), :])

        # scalar queue: Q-proj inputs
        h_t = [cons.tile([128, S], bf16, tag=f"h{i}", name=f"h{i}") for i in range(2)]
        for i in range(2):
            nc.scalar.dma_start(h_t[i][:], hS[128 * i:128 * (i + 1), :])
        wq_t = [cons.tile([128, KVD], bf16, tag=f"wq{i}", name=f"wq{i}") for i in range(2)]
        for i in range(2):
            nc.scalar.dma_start(wq_t[i][:], WqT[128 * i:128 * (i + 1), :])
        bqc_t = cons.tile([128, 4], f32, tag="bqc")
        nc.scalar.dma_start(bqc_t[:], bqc)

        kt_t = [work.tile([128, N], bf16, tag=f"kt{i}", name=f"kt{i}") for i in range(4)]
        v520 = [work.tile([128, 520], bf16, tag=f"v520_{i}", name=f"v520_{i}") for i in range(4)]
        qt_t = [work.tile([128, S], bf16, tag=f"qt{i}", name=f"qt{i}") for i in range(4)]
        att_t = [work.tile([128, S], bf16, tag=f"att{i}", name=f"att{i}") for i in range(4)]
        biasb = work.tile([128, 4], f32, tag="biasb")

        # ACT-table preloads (rsqrt + exp) on tiny const, before the chain
        dum = work.tile([1, 1], f32, tag="dum")
        nc.scalar.activation(dum[:], ones1_t[:], AF.Sqrt)
        nc.scalar.activation(dum[:], ones1_t[:], AF.Exp)

        with tc.tile_pool(name="psSel", bufs=2, space="PSUM") as psSel, \
             tc.tile_pool(name="psM", bufs=3, space="PSUM") as psM:
            # === selector chain (emitted first on each engine) ===
            PTp = psSel.tile([RD, N], f32, tag="psSel", name="PTp")
            for cc in range(4):
                nc.tensor.matmul(PTp[:], wr_t[cc][:], ct_t[cc][:],
                                 start=(cc == 0), stop=(cc == 3))
            PT_sb = work.tile([RD, N], f32, tag="PT_sb")
            nc.vector.tensor_scalar(PT_sb[:], PTp[:], brkc_t[:], None, op0=A.add)
            sq = work.tile([RD, N], f32, tag="sq")
            nc.vector.tensor_tensor(sq[:], PT_sb[:], PT_sb[:], op=A.mult)

            # PE filler while DVE computes sq: first half of K-proj
            kps = []
            for kv in range(2):
                ps = psM.tile([128, N], f32, tag="psM", name=f"kps{kv}")
                for cc in range(4):
                    nc.tensor.matmul(ps[:], wk_t[cc][:, 128 * kv:128 * (kv + 1)],
                                     ctb_t[cc][:], start=(cc == 0), stop=(cc == 3))
                kps.append(ps)

            nrm2p = psSel.tile([1, N], f32, tag="psSel", name="nrm2p")
            nc.tensor.matmul(nrm2p[:], onesd_t[:], sq[:], start=True, stop=True)
            nrm2i = work.tile([1, N], f32, tag="nrm2i")
            nc.vector.reciprocal_approx_fast(nrm2i[:], nrm2p[:])
            rn = work.tile([1, N], f32, tag="rn")
            nc.scalar.activation(rn[:], nrm2i[:], AF.Sqrt)
            rnB = work.tile([RD, N], f32, tag="rnB")
            nc.gpsimd.partition_broadcast(rnB[:], rn[:])
            phatT = work.tile([RD, N], f32, tag="phatT")
            nc.vector.tensor_tensor(phatT[:], PT_sb[:], rnB[:], op=A.mult)
            Scol = work.tile([RD, 1], f32, tag="Scol")
            nc.vector.reduce_sum(Scol[:], phatT[:], axis=mybir.AxisListType.X)

            # PE filler: second half of K-proj
            for kv in range(2, 4):
                ps = psM.tile([128, N], f32, tag="psM", name=f"kps{kv}")
                for cc in range(4):
                    nc.tensor.matmul(ps[:], wk_t[cc][:, 128 * kv:128 * (kv + 1)],
                                     ctb_t[cc][:], start=(cc == 0), stop=(cc == 3))
                kps.append(ps)

            up = psSel.tile([1, N], f32, tag="psSel", name="up")
            nc.tensor.matmul(up[:], Scol[:], phatT[:], start=True, stop=True)
            u_sb = work.tile([1, N], f32, tag="u_sb")
            nc.vector.tensor_copy(u_sb[:], up[:])
            cB = work.tile([128, N], f32, tag="cB")
            nc.gpsimd.partition_broadcast(cB[:], u_sb[:])
            pst = psSel.tile([128, 4], f32, tag="psSel", name="pst")
            for j in range(4):
                nc.tensor.transpose(pst[:, j:j + 1], u_sb[0:1, 128 * j:128 * (j + 1)],
                                    ones1_t[:])
            c4T = work.tile([128, 4], f32, tag="c4T")
            nc.vector.tensor_copy(c4T[:], pst[:])
            cmpd = work.tile([128, N], f32, tag="cmpd")
            rank4 = work.tile([128, 4], f32, tag="rank4")
            for j in range(4):
                nc.vector.tensor_scalar(cmpd[:], cB[:], c4T[:, j:j + 1], 0.0,
                                        op0=A.is_gt, op1=A.add,
                                        accum_out=rank4[:, j:j + 1])
            selm = work.tile([128, 4], f32, tag="selm")
            nc.vector.tensor_scalar(selm[:], rank4[:], 306.5, None, op0=A.is_lt)
            allowed4 = work.tile([128, 4], f32, tag="allowed4")
            nc.vector.tensor_tensor(allowed4[:], selm[:], maskc_t[:], op=A.mult)
            nc.vector.tensor_scalar(biasb[:], allowed4[:], NEGB, NEGB,
                                    op0=A.mult, op1=A.subtract)

            # === projections ===
            # K evacuations on ACT (copy shares every table - no reload)
            for kv in range(4):
                nc.scalar.copy(kt_t[kv][:], kps[kv][:])

            # Q-proj: PE MMs; evacs on DVE right after biasb (kv0 first)
            qps = {}
            for kv in range(4):
                for sc in range(2):
                    ps = psM.tile([128, 512], f32, tag="psM", name=f"qps{kv}_{sc}")
                    for cc in range(2):
                        nc.tensor.matmul(ps[:], wq_t[cc][:, 128 * kv:128 * (kv + 1)],
                                         h_t[cc][:, 512 * sc:512 * (sc + 1)],
                                         start=(cc == 0), stop=(cc == 1))
                    qps[(kv, sc)] = ps
                if kv == 0:
                    for sc in range(2):
                        nc.vector.tensor_scalar(qt_t[0][:, 512 * sc:512 * (sc + 1)],
                                                qps.pop((0, sc))[:],
                                                bqc_t[:, 0:1], None, op0=A.add)

            # V-proj: ones columns early on Pool (idle window), MMs on PE,
            # bias evac on DVE between qt0 and the rest of qt
            for hh in range(NH):
                for nn_ in range(4):
                    nc.gpsimd.tensor_copy(v520[nn_][:, 65 * hh + 64:65 * hh + 65],
                                          onesr_t[:])
            vps = []
            for nn_ in range(4):
                ps = psM.tile([128, KVD], f32, tag="psM", name=f"vps{nn_}")
                for cc in range(4):
                    nc.tensor.matmul(ps[:], ctb_t[cc][:, 128 * nn_:128 * (nn_ + 1)],
                                     wv_t[cc][:], start=(cc == 0), stop=(cc == 3))
                vps.append(ps)
            for nn_ in range(4):
                vview = v520[nn_][:].rearrange("p (h c) -> p h c", c=65)[:, :, 0:64]
                nc.vector.tensor_tensor(vview,
                                        vps[nn_][:].rearrange("p (h c) -> p h c", c=64),
                                        bvB_t[:].rearrange("p (h c) -> p h c", c=64),
                                        op=A.add)
            for kv in range(1, 4):
                for sc in range(2):
                    nc.vector.tensor_scalar(qt_t[kv][:, 512 * sc:512 * (sc + 1)],
                                            qps.pop((kv, sc))[:],
                                            bqc_t[:, kv:kv + 1], None, op0=A.add)

        # === attention + out-projection (shared PSUM block) ===
        with tc.tile_pool(name="psS", bufs=2, space="PSUM") as psS, \
             tc.tile_pool(name="psA", bufs=2, space="PSUM") as psA, \
             tc.tile_pool(name="psB", bufs=2, space="PSUM") as psB:
            dstage = [work.tile([4, 512], bf16, tag=f"dstage{g}", name=f"dstage{g}")
                      for g in range(4)]
            plists = {}
            o65_l = {}

            def scores_exp(h):
                i2 = h // 2
                po = (h % 2) * 64
                plist = []
                for nn_ in range(4):
                    sps_ = psS.tile([128, 1024], f32, tag="psS", name=f"s_{h}_{nn_}")
                    for qc in range(2):
                        nc.tensor.matmul(sps_[:, 512 * qc:512 * (qc + 1)],
                                         kt_t[i2][po:po + 64, 128 * nn_:128 * (nn_ + 1)],
                                         qt_t[i2][po:po + 64, 512 * qc:512 * (qc + 1)],
                                         start=True, stop=True)
                    p_t = ppool.tile([128, 1024], bf16, tag="p", name=f"p_{h}_{nn_}")
                    nc.scalar.activation(p_t[:], sps_[:], AF.Exp,
                                         bias=biasb[:, nn_:nn_ + 1], scale=0.125)
                    plist.append(p_t)
                plists[h] = plist

            def attnv_tail(h):
                i2 = h // 2
                po = (h % 2) * 64
                g = h // 2
                plist = plists.pop(h)
                for qc in range(2):
                    aug = psA.tile([65, 512], f32, tag="psA", name=f"aug_{h}_{qc}")
                    for nn_ in range(4):
                        nc.tensor.matmul(aug[:], v520[nn_][:, 65 * h:65 * h + 65],
                                         plist[nn_][:, 512 * qc:512 * (qc + 1)],
                                         start=(nn_ == 0), stop=(nn_ == 3))
                    o65 = o65p.tile([65, 512], bf16, tag="o65", name=f"o65_{h}_{qc}")
                    nc.vector.tensor_copy(o65[:], aug[:])
                    slot = 2 * (h % 2) + qc
                    nc.sync.dma_start(dstage[g][slot:slot + 1, :], o65[64:65, :])
                    o65_l[(h, qc)] = o65
                if h % 2 == 1:
                    dstF = work.tile([4, 512], f32, tag=f"dstF{g}", name=f"dstF{g}")
                    nc.vector.tensor_copy(dstF[:], dstage[g][:])
                    drecF = work.tile([4, 512], f32, tag=f"drecF{g}", name=f"drecF{g}")
                    nc.vector.reciprocal_approx_fast(drecF[:], dstF[:])
                    drecB = work.tile([4, 512], bf16, tag=f"drecB{g}", name=f"drecB{g}")
                    nc.vector.tensor_copy(drecB[:], drecF[:])
                    for hh in (h - 1, h):
                        i2h = hh // 2
                        poh = (hh % 2) * 64
                        for qc in range(2):
                            slot = 2 * (hh % 2) + qc
                            rrow = rpsp.tile([1, 512], bf16, tag="rrow",
                                             name=f"rrow_{hh}_{qc}")
                            nc.sync.dma_start(rrow[:], drecB[slot:slot + 1, :])
                            rps = rpsp.tile([64, 512], bf16, tag="rps",
                                            name=f"rps_{hh}_{qc}")
                            nc.gpsimd.partition_broadcast(rps[:], rrow[:])
                            nc.vector.tensor_tensor(
                                att_t[i2h][poh:poh + 64, 512 * qc:512 * (qc + 1)],
                                o65_l.pop((hh, qc))[0:64, :], rps[:], op=A.mult)

            scores_exp(0)
            for h in range(1, NH):
                scores_exp(h)
                attnv_tail(h - 1)
            attnv_tail(NH - 1)

            # out-projection: kv0-2 of the first two chains overlap the last
            # pair's divide chain; kv3 only needs att pair 3
            outF = [work.tile([128, S], f32, tag=f"outF{i}", name=f"outF{i}")
                    for i in range(2)]
            dma_eng = [nc.sync, nc.gpsimd, nc.scalar, nc.sync]
            obufs = {}
            for c, (ccn, sc) in enumerate([(0, 0), (0, 1)]):
                ps = psB.tile([128, 512], f32, tag="psB", name=f"ops{ccn}_{sc}")
                for kv in range(3):
                    nc.tensor.matmul(ps[:], wo_t[kv][:, 128 * ccn:128 * (ccn + 1)],
                                     att_t[kv][:, 512 * sc:512 * (sc + 1)],
                                     start=(kv == 0), stop=False)
                obufs[(ccn, sc)] = ps
            for c, (ccn, sc) in enumerate([(0, 0), (0, 1)]):
                ps = obufs.pop((ccn, sc))
                nc.tensor.matmul(ps[:], wo_t[3][:, 128 * ccn:128 * (ccn + 1)],
                                 att_t[3][:, 512 * sc:512 * (sc + 1)],
                                 start=False, stop=True)
                nc.vector.tensor_scalar(outF[ccn][:, 512 * sc:512 * (sc + 1)],
                                        ps[:], boc_t[:, ccn:ccn + 1], None, op0=A.add)
                dma_eng[c].dma_start(y[128 * ccn:128 * (ccn + 1),
                                      512 * sc:512 * (sc + 1)],
                                    outF[ccn][:, 512 * sc:512 * (sc + 1)])
            for c, (ccn, sc) in enumerate([(1, 0), (1, 1)]):
                ps = psB.tile([128, 512], f32, tag="psB", name=f"ops{ccn}_{sc}")
                for kv in range(4):
                    nc.tensor.matmul(ps[:], wo_t[kv][:, 128 * ccn:128 * (ccn + 1)],
                                     att_t[kv][:, 512 * sc:512 * (sc + 1)],
                                     start=(kv == 0), stop=(kv == 3))
                nc.vector.tensor_scalar(outF[ccn][:, 512 * sc:512 * (sc + 1)],
                                        ps[:], boc_t[:, ccn:ccn + 1], None, op0=A.add)
                dma_eng[2 + c].dma_start(y[128 * ccn:128 * (ccn + 1),
                                          512 * sc:512 * (sc + 1)],
                                        outF[ccn][:, 512 * sc:512 * (sc + 1)])

    nc.compile()
    return nc


def _get_nc():
    if "nc" not in _cache:
        _cache["nc"] = _build()
    return _cache["nc"]


def make_in_maps(**inputs):
    import ml_dtypes
    bf = ml_dtypes.bfloat16
    h = np.asarray(inputs["h"], np.float32)
    cond = np.asarray(inputs["cond_feats"], np.float32)
    cmask = np.asarray(inputs["cond_mask"])
    f = np.float32
    shared = {
        "WqT": np.ascontiguousarray(np.asarray(inputs["Wq"], f).T).astype(bf),
        "WkT": np.ascontiguousarray(np.asarray(inputs["Wk"], f).T).astype(bf),
        "WvT": np.ascontiguousarray(np.asarray(inputs["Wv"], f).T).astype(bf),
        "WoT": np.ascontiguousarray(np.asarray(inputs["Wo"], f).T).astype(bf),
        "WrkT": np.ascontiguousarray(np.asarray(inputs["Wrk"], f).T),
        "bqc": np.ascontiguousarray(np.asarray(inputs["bq"], f).reshape(4, 128).T),
        "bvB": np.ascontiguousarray(np.broadcast_to(np.asarray(inputs["bv"], f), (128, KVD))),
        "brkc": np.ascontiguousarray(np.asarray(inputs["brk"], f).reshape(RD, 1)),
        "boc": np.ascontiguousarray(np.asarray(inputs["bo"], f).reshape(2, 128).T),
        "onesd": np.ones((RD, 1), f),
        "ones1": np.ones((1, 1), f),
        "onesr": np.ones((128, 1), bf),
    }
    in_maps = []
    for b in range(B):
        m = dict(shared)
        m["hS"] = np.ascontiguousarray(h[b].reshape(C, S)).astype(bf)
        m["condT"] = np.ascontiguousarray(cond[b].T)
        m["condTb"] = m["condT"].astype(bf)
        m["maskc"] = np.ascontiguousarray(cmask[b].astype(f).reshape(4, 128).T)
        in_maps.append(m)
    return in_maps


def kernel(**inputs):
    from concourse.bass_utils import run_bass_kernel_spmd
    nc = _get_nc()
    in_maps = make_in_maps(**inputs)
    res = run_bass_kernel_spmd(nc, in_maps, core_ids=list(range(N_CORES)))
    return np.stack([res.results[b]["y"].reshape(C, H, W) for b in range(B)])
